# revision 5
# baseline (speedup 1.0000x reference)
"""BiGRU LM kernel for 8 trn2 NeuronCores.

Sharding: vocab-parallel logits/log-softmax (V split 8 x 6283 rows, zero-padded
to 50264), GRU replicated on every core. One AllReduce of the per-position
sum-exp (16 KB) provides the global log-softmax normalizer; the zero-padded V
rows contribute exactly exp(0)=1 each to core 7's sums, corrected by
subtracting PAD_COLS before the log.

No max-subtraction is needed: |h|<1 and |V|<0.089 bound |logit| < 22.6, so
exp() cannot overflow in f32.
"""

import numpy as np
import ml_dtypes

import concourse.bass as bass
import concourse.tile as tile
from concourse import mybir, bacc
from concourse.masks import make_identity

L, B, EMB, REC = 128, 32, 512, 128
VOCAB = 50257
NCORES = 8
VS = 6283                      # vocab shard per core
VPAD = VS * NCORES             # 50264
PAD_COLS = VPAD - VOCAB        # 7 (all on core 7)
NPOS = L * B                   # 4096
NTILE = NPOS // 128            # 32 token tiles
NPB = 32                       # position blocks of 128 for the logits passes
EWIDTH = 1024                  # logits tile width (2 psum banks)
NVT = 7                        # ceil(6283/1024); last tile = 139
LAST_W = VS - (NVT - 1) * EWIDTH  # 139

BF = mybir.dt.bfloat16
F32 = mybir.dt.float32
I32 = mybir.dt.int32
AF = mybir.ActivationFunctionType
ALU = mybir.AluOpType

# bias column indices in the BIAS[128, 8] constant
B_RF, B_IF, B_RB, B_IB, B_NF, B_NB, B2NF, B2NB = range(8)


def build(phases=("front", "rec", "pass1", "ar", "pass2")):
    nc = bacc.Bacc(num_swdge_queues=4)

    idx_p = nc.declare_dram_parameter("idx", [128, NTILE], I32, isOutput=False)
    emb_p = nc.declare_dram_parameter("emb", [VOCAB, EMB], BF, isOutput=False)
    ut_p = nc.declare_dram_parameter("ut", [EMB, 768], BF, isOutput=False)
    wt_p = nc.declare_dram_parameter("wt", [REC, 768], F32, isOutput=False)
    bias_p = nc.declare_dram_parameter("bias", [128, 8], F32, isOutput=False)
    vt_p = nc.declare_dram_parameter("vt", [2 * REC, VS], BF, isOutput=False)
    out_p = nc.declare_dram_parameter("out", [NPOS, VS], F32, isOutput=True)
    nls_p = nc.declare_dram_parameter("nls", [128, NPB], F32, isOutput=True)

    cc_in = nc.dram_tensor("cc_in", [128, NPB], F32)
    cc_out = nc.dram_tensor("cc_out", [128, NPB], F32)

    with tile.TileContext(nc) as tc:
        from contextlib import ExitStack

        with ExitStack() as ctx:
            cpool = ctx.enter_context(tc.tile_pool(name="consts", bufs=1))
            gipool = ctx.enter_context(tc.tile_pool(name="gi", bufs=1))
            hpool = ctx.enter_context(tc.tile_pool(name="hist", bufs=1))

            idx_sb = cpool.tile([128, NTILE], I32)
            ident = cpool.tile([128, 128], BF)
            BIAS = cpool.tile([128, 8], F32)
            W_sb = cpool.tile([128, 768], F32)
            UT_sb = cpool.tile([128, 4, 768], BF)
            VT_sb = cpool.tile([128, 2, VS], BF)

            nc.sync.dma_start(idx_sb[:], idx_p[:, :])
            nc.sync.dma_start(BIAS[:], bias_p[:, :])
            nc.sync.dma_start(W_sb[:], wt_p[:, :])
            ut_src = ut_p[:, :].rearrange("(c p) f -> p c f", p=128)
            nc.sync.dma_start(UT_sb[:], ut_src)
            vt_src = vt_p[:, :].rearrange("(c p) f -> p c f", p=128)
            nc.sync.dma_start(VT_sb[:], vt_src)
            make_identity(nc, ident[:])

            # GI layout: per step t, [r_f i_f r_b i_b] batches of 32; GIN: [n_f n_b]
            GI = gipool.tile([128, L, 4, B], BF)    # 4 MB
            GIN = gipool.tile([128, L, 2, B], BF)   # 2 MB
            SUMS = cpool.tile([128, NPB * 8], F32)
            nc.vector.memset(SUMS[:], 0.0)

            # H32: [fwd 4096 | bwd 4096] f32 recurrence state/history
            H32 = hpool.tile([128, 2 * NPOS], F32)
            H_bf = hpool.tile([128, 2, NPOS], BF)
            nc.vector.memset(H32[:, 0:B], 0.0)                      # fwd h_0
            nc.vector.memset(H32[:, NPOS + (L - 1) * B:2 * NPOS], 0.0)  # bwd col 127

            # ---------------- front: gather -> transpose -> GI ----------------
            # chunk order from both ends so the recurrence's early steps
            # (fwd t=0.., bwd t=127..) have their GI ready first
            chunk_order = [0, 7, 1, 6, 2, 5, 3, 4] if "front" in phases else []
            # gate order in ut columns: [r_f i_f n_f r_b i_b n_b] * 128
            # maps: (GI group, bias col) per gate index
            gate_dst = [
                ("GI", 0, B_RF), ("GI", 1, B_IF), ("GIN", 0, B_NF),
                ("GI", 2, B_RB), ("GI", 3, B_IB), ("GIN", 1, B_NB),
            ]
            with (
                tc.tile_pool(name="front", bufs=4) as fpool,
                tc.tile_pool(name="et", bufs=1) as etpool,
                tc.tile_pool(name="pst", bufs=4, space="PSUM") as pst,
                tc.tile_pool(name="psg", bufs=3, space="PSUM") as psg,
            ):
                ET = etpool.tile([128, 4, NPOS], BF)  # embs.T, 4 EMB chunks
                for ch in chunk_order:
                    for jj in range(4):
                        jt = ch * 4 + jj
                        et = fpool.tile([128, EMB], BF, tag="embtile")
                        nc.gpsimd.indirect_dma_start(
                            out=et[:],
                            out_offset=None,
                            in_=emb_p[:, :],
                            in_offset=bass.IndirectOffsetOnAxis(
                                ap=idx_sb[:, jt:jt + 1], axis=0
                            ),
                        )
                        for kc in range(4):
                            pt = pst.tile([128, 128], BF)
                            nc.tensor.transpose(
                                pt[:], et[:, kc * 128:(kc + 1) * 128], ident[:]
                            )
                            nc.vector.tensor_copy(
                                ET[:, kc, jt * 128:(jt + 1) * 128], pt[:]
                            )
                    # GI matmuls for this 512-token chunk
                    t0 = ch * 16  # first step index of chunk (512 tokens = 16 steps)
                    for g in range(6):
                        ps = psg.tile([128, 512], F32)
                        for kc in range(4):
                            nc.tensor.matmul(
                                ps[:],
                                UT_sb[:, kc, g * 128:(g + 1) * 128],
                                ET[:, kc, ch * 512:(ch + 1) * 512],
                                start=(kc == 0),
                                stop=(kc == 3),
                            )
                        grp, gi, bcol = gate_dst[g]
                        if grp == "GI":
                            dst = GI[:, t0:t0 + 16, gi, :]
                        else:
                            dst = GIN[:, t0:t0 + 16, gi, :]
                        nc.scalar.activation(
                            dst, ps[:].rearrange("p (t b) -> p t b", b=B),
                            AF.Identity, bias=BIAS[:, bcol:bcol + 1],
                        )

            # ---------------- recurrence (127 steps, fwd+bwd fused) -----------
            with (
                tc.tile_pool(name="dsmall", bufs=3) as dpool,
                tc.tile_pool(name="psd", bufs=3, space="PSUM") as psd,
            ):
                for s in range(L - 1 if "rec" in phases else 0):
                    tb = L - 1 - s  # bwd token/step index
                    hf = H32[:, s * B:(s + 1) * B]
                    hb = H32[:, NPOS + tb * B:NPOS + (tb + 1) * B]
                    ps = psd.tile([128, 192], F32)
                    # col blocks: [r_f i_f r_b i_b n_f n_b]
                    for j, (gcol, h) in enumerate(
                        [(0, hf), (1, hf), (3, hb), (4, hb), (2, hf), (5, hb)]
                    ):
                        nc.tensor.matmul(
                            ps[:, j * B:(j + 1) * B],
                            W_sb[:, gcol * 128:(gcol + 1) * 128],
                            h,
                        )
                    # t_ri = gh_ri + gi_ri   [128, 128]
                    gi_f = GI[:, s, 0:2, :]
                    gi_b = GI[:, tb, 2:4, :]
                    t_ri = dpool.tile([128, 128], F32, tag="tri")
                    nc.vector.tensor_add(t_ri[:, 0:64], ps[:, 0:64], gi_f)
                    nc.vector.tensor_add(t_ri[:, 64:128], ps[:, 64:128], gi_b)
                    rz = dpool.tile([128, 128], F32, tag="rz")
                    nc.scalar.activation(rz[:], t_ri[:], AF.Sigmoid)
                    # q = z * h_prev  (z cols: 32:64 fwd, 96:128 bwd)
                    q = dpool.tile([128, 64], F32, tag="q")
                    nc.vector.tensor_mul(q[:, 0:32], rz[:, 32:64], hf)
                    nc.vector.tensor_mul(q[:, 32:64], rz[:, 96:128], hb)
                    # t1 = (gh_n + b2n) * r
                    t1 = dpool.tile([128, 64], F32, tag="t1")
                    nc.vector.scalar_tensor_tensor(
                        t1[:, 0:32], ps[:, 128:160], BIAS[:, B2NF:B2NF + 1],
                        rz[:, 0:32], op0=ALU.add, op1=ALU.mult,
                    )
                    nc.vector.scalar_tensor_tensor(
                        t1[:, 32:64], ps[:, 160:192], BIAS[:, B2NB:B2NB + 1],
                        rz[:, 64:96], op0=ALU.add, op1=ALU.mult,
                    )
                    # t2 = t1 + gi_n
                    t2 = dpool.tile([128, 64], F32, tag="t2")
                    nc.vector.tensor_add(t2[:, 0:32], t1[:, 0:32], GIN[:, s, 0, :])
                    nc.vector.tensor_add(t2[:, 32:64], t1[:, 32:64], GIN[:, tb, 1, :])
                    n = dpool.tile([128, 64], F32, tag="n")
                    nc.scalar.activation(n[:], t2[:], AF.Tanh)
                    # u = (z - 1) * n ; h' = q - u
                    u = dpool.tile([128, 64], F32, tag="u")
                    nc.vector.scalar_tensor_tensor(
                        u[:, 0:32], rz[:, 32:64], 1.0, n[:, 0:32],
                        op0=ALU.subtract, op1=ALU.mult,
                    )
                    nc.vector.scalar_tensor_tensor(
                        u[:, 32:64], rz[:, 96:128], 1.0, n[:, 32:64],
                        op0=ALU.subtract, op1=ALU.mult,
                    )
                    hf_new = H32[:, (s + 1) * B:(s + 2) * B]
                    hb_new = H32[:, NPOS + (tb - 1) * B:NPOS + tb * B]
                    nc.vector.tensor_sub(hf_new, q[:, 0:32], u[:, 0:32])
                    nc.vector.tensor_sub(hb_new, q[:, 32:64], u[:, 32:64])

            # cast recurrence history to bf16 for the logits matmuls
            nc.vector.tensor_copy(H_bf[:, 0, :], H32[:, 0:NPOS])
            nc.vector.tensor_copy(H_bf[:, 1, :], H32[:, NPOS:2 * NPOS])

            # ---------------- pass 1: sum-exp over the vocab shard ------------
            with (
                tc.tile_pool(name="scr", bufs=3) as scrpool,
                tc.tile_pool(name="pse", bufs=3, space="PSUM") as pse,
            ):
                for pb in range(NPB if "pass1" in phases else 0):
                    for vt in range(NVT):
                        w = LAST_W if vt == NVT - 1 else EWIDTH
                        c0 = vt * EWIDTH
                        ps = pse.tile([128, EWIDTH], F32)
                        for half in range(0, w, 512):
                            hw = min(512, w - half)
                            for k in range(2):
                                nc.tensor.matmul(
                                    ps[:, half:half + hw],
                                    H_bf[:, k, pb * 128:(pb + 1) * 128],
                                    VT_sb[:, k, c0 + half:c0 + half + hw],
                                    start=(k == 0),
                                    stop=(k == 1),
                                )
                        scr = scrpool.tile([128, EWIDTH], BF, tag="scr")
                        nc.scalar.activation(
                            scr[:, 0:w], ps[:, 0:w], AF.Exp,
                            accum_out=SUMS[:, pb * 8 + vt:pb * 8 + vt + 1],
                        )

            # ---------------- normalizer: AllReduce + log ---------------------
            S_all = cpool.tile([128, NPB], F32)
            nc.vector.tensor_reduce(
                S_all[:],
                SUMS[:].rearrange("p (a b) -> p a b", b=8),
                axis=mybir.AxisListType.X,
                op=ALU.add,
            )
            nc.sync.dma_start(cc_in[:, :], S_all[:])
            nc.gpsimd.collective_compute(
                "AllReduce",
                ALU.add,
                replica_groups=[list(range(NCORES))],
                ins=[cc_in[:, :].opt()],
                outs=[cc_out[:, :].opt()],
            )
            S_red = cpool.tile([128, NPB], F32)
            nc.sync.dma_start(S_red[:], cc_out[:, :])
            logS = cpool.tile([128, NPB], F32)
            negpad = cpool.tile([128, 1], F32)
            nc.vector.memset(negpad[:], -float(PAD_COLS))
            nc.scalar.activation(logS[:], S_red[:], AF.Ln, bias=negpad[:])
            neg_logS = cpool.tile([128, NPB], F32)
            nc.vector.tensor_scalar_mul(neg_logS[:], logS[:], -1.0)
            nc.sync.dma_start(nls_p[:, :], neg_logS[:])

            # ---------------- pass 2: logits - logS -> out --------------------
            with (
                tc.tile_pool(name="stage", bufs=2) as stpool,
                tc.tile_pool(name="ps2", bufs=4, space="PSUM") as ps2,
            ):
                for pb in range(NPB if "pass2" in phases else 0):
                    stg = stpool.tile([128, VS], F32, tag="stage")
                    for vt in range(NVT):
                        w = LAST_W if vt == NVT - 1 else EWIDTH
                        c0 = vt * EWIDTH
                        ps = ps2.tile([128, EWIDTH], F32)
                        for half in range(0, w, 512):
                            hw = min(512, w - half)
                            for k in range(2):
                                nc.tensor.matmul(
                                    ps[:, half:half + hw],
                                    H_bf[:, k, pb * 128:(pb + 1) * 128],
                                    VT_sb[:, k, c0 + half:c0 + half + hw],
                                    start=(k == 0),
                                    stop=(k == 1),
                                )
                        if vt % 2 == 0:
                            nc.scalar.activation(
                                stg[:, c0:c0 + w], ps[:, 0:w], AF.Identity,
                                bias=neg_logS[:, pb:pb + 1],
                            )
                        else:
                            nc.vector.tensor_scalar_add(
                                stg[:, c0:c0 + w], ps[:, 0:w],
                                neg_logS[:, pb:pb + 1],
                            )
                    nc.sync.dma_start(out_p[pb * 128:(pb + 1) * 128, :], stg[:])

    nc.finalize()
    return nc


_cache = {}


def _get_nc():
    if "nc" not in _cache:
        _cache["nc"] = build()
    return _cache["nc"]


def _host_prep(inputs):
    bf16 = ml_dtypes.bfloat16
    idx = np.ascontiguousarray(
        inputs["input_batch"].astype(np.int32).reshape(NPOS).reshape(NTILE, 128).T
    )
    emb_bf = inputs["embedding"].astype(bf16)
    ut = np.ascontiguousarray(
        np.concatenate([inputs["U"], inputs["U_b"]], axis=0).T
    ).astype(bf16)  # [512, 768]
    wt = np.ascontiguousarray(
        np.concatenate([inputs["W"], inputs["W_b"]], axis=0).T
    ).astype(np.float32)  # [128, 768]

    b1, b2 = inputs["bias_1"], inputs["bias_2"]
    b1b, b2b = inputs["bias_1_b"], inputs["bias_2_b"]
    bias = np.zeros((128, 8), np.float32)
    bias[:, B_RF] = b1[0:128] + b2[0:128]
    bias[:, B_IF] = b1[128:256] + b2[128:256]
    bias[:, B_RB] = b1b[0:128] + b2b[0:128]
    bias[:, B_IB] = b1b[128:256] + b2b[128:256]
    bias[:, B_NF] = b1[256:384]
    bias[:, B_NB] = b1b[256:384]
    bias[:, B2NF] = b2[256:384]
    bias[:, B2NB] = b2b[256:384]

    vt_full = np.zeros((2 * REC, VPAD), np.float32)
    vt_full[:, :VOCAB] = inputs["V"].T
    vt_bf = vt_full.astype(bf16)

    in_maps = []
    for c in range(NCORES):
        in_maps.append(
            {
                "idx": idx,
                "emb": emb_bf,
                "ut": ut,
                "wt": wt,
                "bias": bias,
                "vt": np.ascontiguousarray(vt_bf[:, c * VS:(c + 1) * VS]),
            }
        )
    return in_maps


def kernel(**inputs):
    from concourse.bass_utils import run_bass_kernel_spmd

    nc = _get_nc()
    in_maps = _host_prep(inputs)
    res = run_bass_kernel_spmd(nc, in_maps, core_ids=list(range(NCORES)))
    out = np.empty((NPOS, VPAD), np.float32)
    for c in range(NCORES):
        out[:, c * VS:(c + 1) * VS] = res.results[c]["out"]
    return out[:, :VOCAB].reshape(L, B, VOCAB)
